# revision 25
# baseline (speedup 1.0000x reference)
"""Trainium2 Bass kernel for nn_Basic_Model_19078244729512.

Computes per-sample "returning rate" vectors p1, p2 from a [B, 25] grid
(reshaped [B, 5, 5]) of probabilities plus a mask tensor.

The f32 full-row baseline (155-157 us) ran at the HBM roofline for
54 MB/core of traffic.  This version cuts device HBM traffic ~4x (to
13.5 MB/core: 38 B/row in + 16 B/row out) by observing that the
computation reads only 15 of 25 `output` columns and 4 of 25
`label_mask` columns, and that the 2e-2 harness rel-err gate leaves
~18x margin for fp16 end-to-end (measured rel err 1.1e-3):

  * Host packs the 19 needed columns into one fp16 tensor, laid out
    [128 partitions][chunk][19 column-blocks][F samples] per core so
    every device-side operand is a unit-stride fp16 block (DVE 2x mode;
    even F keeps blocks 4 B-aligned) and each input DMA moves
    per-partition-contiguous 19*F*2 B spans.
  * All device compute is fp16.  p1: masked-head products h=L*M,
    pm=h*R (pm0 is p1_1 directly), three broadcast chain multiplies,
    then in-place shifted adds so pv[4:8] ends as [p1_1..p1_4] and is
    DMA'd straight out.  p2: q=1-p on ACT, 3-step cumprod on DVE,
    V=1-S on ACT (1-S1 folds to the raw p40 column), o2=V*QM.
  * Device writes fp16; the constant-zero column 0 and the upcast to
    fp32 happen host-side during the gather.
  * Chunks are (492,492,492,478) samples/partition (sum 1954), so the
    global batch pads only 2,000,000 -> 2,000,896 (0.045%).

After the traffic cut the kernel is DVE-bound, at the algebraic floor
of 31 F-blocks of fp16 tensor_tensor work (~32 us/pass + op overhead;
measured 35-40 us depending on axon tenancy; pure-DMA ablation of the
same traffic measured 23-33 us).  Measured/modeled dead ends from this
session, kept out of the code:

  * GPSIMD tensor_tensor offload of the adds or QM: +8..13 us (Q7 SW
    ops are ~4x slower than DVE 2x and contend with SWDGE output-DMA
    emission).
  * scalar_tensor_tensor runs at DVE 1x (2110 ns vs 1085 ns for a 4F
    tensor_tensor), which kills uint8 inputs: the dequant scales must
    ride on stt muls (2x cost) or a separate 19F scaling pass (+10 us).
  * DMA accum_op=mult (compute u-products during the input DMA) fails
    at walrus compile; accum_op=add works but HBM-side accumulate costs
    more extra traffic than the 6F of adds it would save.
  * Custom DVE Specs run at 1x for fp16 (stock tensor_tensor 2x beats
    any fused form for pairwise products).
  * ACT cannot run tensor_tensor (activation-only engine); TensorE
    cannot form elementwise products.  F=984 with tmp_bufs=1 serializes
    the pipeline (62 us); F=656/bufs=4 are within noise of F=492.
  * Scratch tiles reuse spent blocks (u1 overwrites h after pm consumes
    it; QM overwrites q31..q04 after the chain) so even 2-chunk
    (978,976) fits SBUF with full double-buffering -- but 2/3/4-chunk
    splits all measure within noise (min-est 37.7/38.5/38.6 us), so the
    per-op overhead saving of fewer, larger ops is offset by coarser
    pipelining.  Stride-0 broadcast operands do NOT drop the DVE to 1x
    (no_bcast diagnostic within noise of control), and splitting the
    input DMA across both HWDGE rings (in_split) changes nothing.
  * Outputs go out on the scalar HWDGE ring, not gpsimd SWDGE: with the
    kernel DVE-bound, SWDGE's descriptor rings (SBUF partitions 0-31)
    contend with DVE SBUF traffic, and HWDGE-out measured consistently
    ~2-4 us faster (min-est 34.2/36.9 vs 41.5/39.3 for SWDGE across two
    interleaved rounds) -- the reverse of the first session's finding
    on the DMA-bound f32 kernel, where SWDGE-out won by ~11 us.
"""

import numpy as np

_B = 2_000_000
_NCORES = 8
# chunk sizes (samples per partition per chunk): even (fp16 2x-mode 4B
# alignment); sum = 1954 so the global pad is only 896 rows
_CHUNKS = (492, 492, 492, 478)
_FTOT = sum(_CHUNKS)      # 1954 samples per partition per core
_NPC = 128 * _FTOT        # 250112 rows per core
_NC_IN = 19               # packed input columns per sample
_NC_OUT = 8               # output columns per sample (p1[1:5] | p2[1:5])

# packed column order (flat 5x5 index = 5*i+j):
#  0-4   L  = p40 p31 p22 p13 p04       (from `output`)
#  5-8   M  = m31 m22 m13 m04           (from `label_mask`)
#  9-12  R0 = p41 p32 p23 p14           (partners of the masked head terms)
#  13-15 RA = p42 p43 p44               (p40-chain partners)
#  16-17 RB = p33 p34                   (p31-chain partners)
#  18    RC = p24                       (p22-chain partner)
_PRED_COLS = [20, 16, 12, 8, 4]               # -> packed 0..4
_MASK_COLS = [16, 12, 8, 4]                   # -> packed 5..8
_PRED_COLS2 = [21, 17, 13, 9, 22, 23, 24, 18, 19, 14]  # -> packed 9..18


def _legalize_waits(nc):
    """Split multi-wait sync_info into standalone EventSemaphore waits.

    The walrus build in this container encodes at most one sync-wait command
    per ISA instruction ("Too many sync wait commands" otherwise); hoist all
    but the last wait of each instruction into preceding single-wait
    EventSemaphore ops on the same engine (semantically identical: all waits
    are monotone semaphore conditions checked before issue).
    """
    import concourse.mybir as mybir

    for fn in nc.m.functions:
        for blk in fn.blocks:
            out = []
            for inst in blk.instructions:
                si = getattr(inst, "sync_info", None)
                waits = list(si.on_wait) if si is not None and si.on_wait else []
                if len(waits) > 1:
                    for k, w in enumerate(waits[:-1]):
                        out.append(mybir.InstEventSemaphore(
                            name=f"{inst.name}-w{k}",
                            engine=inst.engine,
                            ins=[], outs=[],
                            sync_info=mybir.SyncInfo(on_wait=[w], on_update=[]),
                        ))
                    inst.sync_info = mybir.SyncInfo(
                        on_wait=[waits[-1]],
                        on_update=list(si.on_update) if si.on_update else [],
                    )
                out.append(inst)
            blk.instructions = out
    return nc


def build_nc(reps=1, legalize=True, in_bufs=3, out_bufs=3, tmp_bufs=2,
             out_dma_engine="scalar", in_dma_engine="sync", dma_only=False,
             chunks=None, no_bcast=False, in_split=False):
    import concourse.bass as bass
    import concourse.mybir as mybir
    from concourse.tile import TileContext

    f16 = mybir.dt.float16
    MUL = mybir.AluOpType.mult
    ADD = mybir.AluOpType.add
    COPY = mybir.ActivationFunctionType.Copy
    chunks = chunks or _CHUNKS
    assert sum(chunks) == _FTOT

    nc = bass.Bass("TRN2", target_bir_lowering=False, debug=False)
    x = nc.dram_tensor("xin", [128, _FTOT * _NC_IN], f16, kind="ExternalInput")
    o = nc.dram_tensor("o12", [128, _FTOT * _NC_OUT], f16, kind="ExternalOutput")

    with TileContext(nc) as tc:
        with (
            tc.tile_pool(name="inp", bufs=in_bufs) as inp,
            tc.tile_pool(name="out", bufs=out_bufs) as outp,
            tc.tile_pool(name="tmp", bufs=tmp_bufs) as tmp,
        ):
            engines = {"sync": nc.sync, "gpsimd": nc.gpsimd, "scalar": nc.scalar}
            in_eng = engines[in_dma_engine]
            out_eng = engines[out_dma_engine]

            def emit_chunk(off, F):
                ioff, ooff = _NC_IN * off, _NC_OUT * off
                T = inp.tile([128, _NC_IN * F], f16, tag="tin")
                if in_split:
                    h10 = 10 * F
                    nc.sync.dma_start(T[:, 0:h10], x[:, ioff:ioff + h10])
                    nc.scalar.dma_start(
                        T[:, h10:], x[:, ioff + h10:ioff + _NC_IN * F])
                else:
                    in_eng.dma_start(T[:], x[:, ioff:ioff + _NC_IN * F])
                if dma_only:
                    # ablation: identical HBM traffic, no compute
                    out_eng.dma_start(
                        o[:, ooff:ooff + _NC_OUT * F], T[:, 0:_NC_OUT * F])
                    return
                Tv = T[:].rearrange("p (c f) -> p c f", f=F)

                # pv blocks (11): 0-3 h, reused for u1 once pm consumes h |
                # 4-7 pm (becomes [p1_1..p1_4] in place) | 8-9 u2 | 10 u3
                pv = tmp.tile([128, 11 * F], f16, tag="pv")
                pvv = pv[:].rearrange("p (c f) -> p c f", f=F)
                # qs blocks (8): 0-4 q40..q04 (1-4 become QM in place) | 5-7 S2-4
                qs = tmp.tile([128, 8 * F], f16, tag="qs")
                qv = qs[:].rearrange("p (c f) -> p c f", f=F)
                vq = tmp.tile([128, 3 * F], f16, tag="vq")
                vqv = vq[:].rearrange("p (c f) -> p c f", f=F)
                o2t = outp.tile([128, 4 * F], f16, tag="o2t")
                o2v = o2t[:].rearrange("p (c f) -> p c f", f=F)

                # ---- ACT: q = 1 - [p40 p31 p22 p13 p04] (issued first so it
                # overlaps the DVE p1 head block) ----
                nc.scalar.activation(qv[:, 0:5], Tv[:, 0:5], COPY,
                                     bias=1.0, scale=-1.0)

                # ---- p1 head products (DVE; only need T) ----
                # h = L*M = [p40*m31, p31*m22, p22*m13, p13*m04]
                nc.vector.tensor_tensor(pvv[:, 0:4], Tv[:, 0:4], Tv[:, 5:9], MUL)
                # masked head terms pm = h*[p41 p32 p23 p14]; pm0 = p1_1
                nc.vector.tensor_tensor(pvv[:, 4:8], pvv[:, 0:4], Tv[:, 9:13], MUL)
                # u1 = p40*[p42 p43 p44] over the spent h blocks
                # (no_bcast: timing-only diagnostic for whether stride-0
                # broadcast operands silently drop the DVE to 1x mode)
                u1_rhs = (Tv[:, 0:3] if no_bcast
                          else Tv[:, 0:1].broadcast_to((128, 3, F)))
                nc.vector.tensor_tensor(pvv[:, 0:3], Tv[:, 13:16], u1_rhs, MUL)

                # ---- p2 cumprod chain (ACT q ran during h/pm/u1); issued
                # early so ACT's V op completes long before o2 consumes it ----
                nc.vector.tensor_tensor(qv[:, 5:6], qv[:, 0:1], qv[:, 1:2], MUL)
                nc.vector.tensor_tensor(qv[:, 6:7], qv[:, 5:6], qv[:, 2:3], MUL)
                nc.vector.tensor_tensor(qv[:, 7:8], qv[:, 6:7], qv[:, 3:4], MUL)
                # V = 1 - [S2 S3 S4] on ACT (1-S1 = p40 is a raw input column)
                nc.scalar.activation(vqv[:, 0:3], qv[:, 5:8], COPY,
                                     bias=1.0, scale=-1.0)

                # ---- p1 tail (covers ACT V latency) ----
                # u2 = p31*[p33 p34], u3 = p22*p24
                u2_rhs = (Tv[:, 1:3] if no_bcast
                          else Tv[:, 1:2].broadcast_to((128, 2, F)))
                nc.vector.tensor_tensor(pvv[:, 8:10], Tv[:, 16:18], u2_rhs, MUL)
                nc.vector.tensor_tensor(pvv[:, 10:11], Tv[:, 18:19], Tv[:, 2:3], MUL)
                # p1_j = pm_j + prefix-chain partials via in-place shifted adds;
                # pv[4:8] ends as [p1_1 p1_2 p1_3 p1_4]
                nc.vector.tensor_tensor(pvv[:, 5:8], pvv[:, 5:8], pvv[:, 0:3], ADD)
                nc.vector.tensor_tensor(pvv[:, 6:8], pvv[:, 6:8], pvv[:, 8:10], ADD)
                nc.vector.tensor_tensor(pvv[:, 7:8], pvv[:, 7:8], pvv[:, 10:11], ADD)
                # QM = [q31 q22 q13 q04]*[m31 m22 m13 m04] in place over q
                # (the chain has already consumed q31/q22/q13)
                nc.vector.tensor_tensor(qv[:, 1:5], qv[:, 1:5], Tv[:, 5:9], MUL)
                # p2_1 = (1-S1)*q31*m31 = p40*QM0
                nc.vector.tensor_tensor(o2v[:, 0:1], Tv[:, 0:1], qv[:, 1:2], MUL)
                # p2_j = V_j*QM_j, j=2..4
                nc.vector.tensor_tensor(o2v[:, 1:4], vqv[:, 0:3], qv[:, 2:5], MUL)

                # p1 goes out on the sync(SP) ring, p2 on out_eng: the rings'
                # sequencers block on each DMA's wait-for-DVE sem, so putting
                # both on ACT would delay the next chunk's q activation
                nc.sync.dma_start(o[:, ooff:ooff + 4 * F], pv[:, 4 * F:8 * F])
                out_eng.dma_start(o[:, ooff + 4 * F:ooff + 8 * F], o2t[:])

            for _ in range(reps):
                off = 0
                for F in chunks:
                    emit_chunk(off, F)
                    off += F
    return _legalize_waits(nc) if legalize else nc


def _pack_inputs(output, label_mask):
    """[B,25] f32 x2 -> per-core [128, FTOT*19] fp16 device layout."""
    ntot = _NCORES * _NPC
    xp = np.zeros((ntot, _NC_IN), np.float16)
    xp[:_B, 0:5] = output[:, _PRED_COLS]
    xp[:_B, 5:9] = label_mask[:, _MASK_COLS]
    xp[:_B, 9:19] = output[:, _PRED_COLS2]
    cores = []
    for c in range(_NCORES):
        a = xp[c * _NPC:(c + 1) * _NPC].reshape(128, _FTOT, _NC_IN)
        parts, off = [], 0
        for F in _CHUNKS:
            blk = a[:, off:off + F, :].transpose(0, 2, 1)  # [128, 19, F]
            parts.append(np.ascontiguousarray(blk).reshape(128, _NC_IN * F))
            off += F
        cores.append(np.concatenate(parts, axis=1))
    return cores


def _unpack_outputs(res):
    """Per-core [128, FTOT*8] fp16 -> (p1, p2) [B,5] f32."""
    p1 = np.zeros((_B, 5), np.float32)
    p2 = np.zeros((_B, 5), np.float32)
    rows = np.empty((_NCORES * _NPC, _NC_OUT), np.float32)
    for c in range(_NCORES):
        a = np.asarray(res[c]["o12"])
        parts, off = [], 0
        for F in _CHUNKS:
            blk = a[:, _NC_OUT * off:_NC_OUT * (off + F)]
            parts.append(blk.reshape(128, _NC_OUT, F).transpose(0, 2, 1))
            off += F
        core_rows = np.concatenate(parts, axis=1)  # [128, FTOT, 8]
        rows[c * _NPC:(c + 1) * _NPC] = core_rows.reshape(_NPC, _NC_OUT)
    p1[:, 1:5] = rows[:_B, 0:4]
    p2[:, 1:5] = rows[:_B, 4:8]
    return p1, p2


def _run(output, label_mask, **spmd_kwargs):
    from concourse.bass_utils import run_bass_kernel_spmd

    output = np.ascontiguousarray(np.asarray(output), dtype=np.float32)
    label_mask = np.ascontiguousarray(np.asarray(label_mask), dtype=np.float32)
    assert output.shape == (_B, 25) and label_mask.shape == (_B, 25)

    in_maps = [{"xin": xc} for xc in _pack_inputs(output, label_mask)]
    nc = build_nc()
    bres = run_bass_kernel_spmd(nc, in_maps, list(range(_NCORES)), **spmd_kwargs)
    p1, p2 = _unpack_outputs(bres.results)
    return p1, p2, bres


def kernel(output, label_mask):
    p1, p2, _ = _run(output, label_mask)
    return p1, p2


# revision 28
# speedup vs baseline: 1.4488x; 1.4488x over previous
"""Trainium2 Bass kernel for nn_Basic_Model_19078244729512.

Computes per-sample "returning rate" vectors p1, p2 from a [B, 25] grid
(reshaped [B, 5, 5]) of probabilities plus a mask tensor.

The f32 full-row baseline (155-157 us) ran at the HBM roofline for
54 MB/core of traffic.  This version cuts device HBM traffic ~4x (to
13.5 MB/core: 38 B/row in + 16 B/row out) by observing that the
computation reads only 15 of 25 `output` columns and 4 of 25
`label_mask` columns, and that the 2e-2 harness rel-err gate leaves
~18x margin for fp16 end-to-end (measured rel err 1.1e-3):

  * Host packs the 19 needed columns into one fp16 tensor, laid out
    [128 partitions][chunk][19 column-blocks][F samples] per core so
    every device-side operand is a unit-stride fp16 block (DVE 2x mode;
    even F keeps blocks 4 B-aligned) and each input DMA moves
    per-partition-contiguous 19*F*2 B spans.
  * All device compute is fp16.  p1: masked-head products h=L*M,
    pm=h*R (pm0 is p1_1 directly), three broadcast chain multiplies,
    then in-place shifted adds so pv[4:8] ends as [p1_1..p1_4] and is
    DMA'd straight out.  p2: q=1-p on ACT, 3-step cumprod on DVE,
    V=1-S on ACT (1-S1 folds to the raw p40 column), o2=V*QM.
  * Device writes fp16; the constant-zero column 0 and the upcast to
    fp32 happen host-side during the gather.
  * Chunks are (492,492,492,478) samples/partition (sum 1954), so the
    global batch pads only 2,000,000 -> 2,000,896 (0.045%).

After the traffic cut the kernel is DVE-bound, at the algebraic floor
of 31 F-blocks of fp16 tensor_tensor work (~32 us/pass + op overhead;
measured 35-40 us depending on axon tenancy; pure-DMA ablation of the
same traffic measured 23-33 us).  Measured/modeled dead ends from this
session, kept out of the code:

  * GPSIMD tensor_tensor offload of the adds or QM: +8..13 us (Q7 SW
    ops are ~4x slower than DVE 2x and contend with SWDGE output-DMA
    emission).
  * scalar_tensor_tensor runs at DVE 1x (2110 ns vs 1085 ns for a 4F
    tensor_tensor), which kills uint8 inputs: the dequant scales must
    ride on stt muls (2x cost) or a separate 19F scaling pass (+10 us).
  * DMA accum_op=mult (compute u-products during the input DMA) fails
    at walrus compile; accum_op=add works but HBM-side accumulate costs
    more extra traffic than the 6F of adds it would save.
  * Custom DVE Specs run at 1x for fp16 (stock tensor_tensor 2x beats
    any fused form for pairwise products).
  * ACT cannot run tensor_tensor (activation-only engine); TensorE
    cannot form elementwise products.  F=984 with tmp_bufs=1 serializes
    the pipeline (62 us); F=656/bufs=4 are within noise of F=492.
  * Scratch tiles reuse spent blocks (u1 overwrites h after pm consumes
    it; QM overwrites q31..q04 after the chain) so even 2-chunk
    (978,976) fits SBUF with full double-buffering -- but 2/3/4-chunk
    splits all measure within noise (min-est 37.7/38.5/38.6 us), so the
    per-op overhead saving of fewer, larger ops is offset by coarser
    pipelining.  Stride-0 broadcast operands do NOT drop the DVE to 1x
    (no_bcast diagnostic within noise of control), and splitting the
    input DMA across both HWDGE rings (in_split) changes nothing.
  * Outputs go out on the HWDGE rings (p1 on sync, p2 on scalar), not
    gpsimd SWDGE: with the kernel DVE-bound, SWDGE's descriptor rings
    (SBUF partitions 0-31) contend with DVE SBUF traffic, and HWDGE-out
    measured consistently ~2-4 us faster (min-est 34.2/36.9 vs
    41.5/39.3 across two interleaved rounds) -- the reverse of the
    first session's finding on the DMA-bound f32 kernel, where
    SWDGE-out won by ~11 us.  Within HWDGE, split-rings vs both-on-
    scalar are equivalent (A/B within noise), but both-on-sync is
    consistently worst: two output waits queued ahead of the input
    DMAs on the SP sequencer delay every following load.
"""

import numpy as np

_B = 2_000_000
_NCORES = 8
# chunk sizes (samples per partition per chunk): even (fp16 2x-mode 4B
# alignment); sum = 1954 so the global pad is only 896 rows
_CHUNKS = (492, 492, 492, 478)
_FTOT = sum(_CHUNKS)      # 1954 samples per partition per core
_NPC = 128 * _FTOT        # 250112 rows per core
_NC_IN = 19               # packed input columns per sample
_NC_OUT = 8               # output columns per sample (p1[1:5] | p2[1:5])

# packed column order (flat 5x5 index = 5*i+j):
#  0-4   L  = p40 p31 p22 p13 p04       (from `output`)
#  5-8   M  = m31 m22 m13 m04           (from `label_mask`)
#  9-12  R0 = p41 p32 p23 p14           (partners of the masked head terms)
#  13-15 RA = p42 p43 p44               (p40-chain partners)
#  16-17 RB = p33 p34                   (p31-chain partners)
#  18    RC = p24                       (p22-chain partner)
_PRED_COLS = [20, 16, 12, 8, 4]               # -> packed 0..4
_MASK_COLS = [16, 12, 8, 4]                   # -> packed 5..8
_PRED_COLS2 = [21, 17, 13, 9, 22, 23, 24, 18, 19, 14]  # -> packed 9..18


def _legalize_waits(nc):
    """Split multi-wait sync_info into standalone EventSemaphore waits.

    The walrus build in this container encodes at most one sync-wait command
    per ISA instruction ("Too many sync wait commands" otherwise); hoist all
    but the last wait of each instruction into preceding single-wait
    EventSemaphore ops on the same engine (semantically identical: all waits
    are monotone semaphore conditions checked before issue).
    """
    import concourse.mybir as mybir

    for fn in nc.m.functions:
        for blk in fn.blocks:
            out = []
            for inst in blk.instructions:
                si = getattr(inst, "sync_info", None)
                waits = list(si.on_wait) if si is not None and si.on_wait else []
                if len(waits) > 1:
                    for k, w in enumerate(waits[:-1]):
                        out.append(mybir.InstEventSemaphore(
                            name=f"{inst.name}-w{k}",
                            engine=inst.engine,
                            ins=[], outs=[],
                            sync_info=mybir.SyncInfo(on_wait=[w], on_update=[]),
                        ))
                    inst.sync_info = mybir.SyncInfo(
                        on_wait=[waits[-1]],
                        on_update=list(si.on_update) if si.on_update else [],
                    )
                out.append(inst)
            blk.instructions = out
    return nc


def build_nc(reps=1, legalize=True, in_bufs=3, out_bufs=3, tmp_bufs=2,
             out_dma_engine="scalar", in_dma_engine="sync", dma_only=False,
             chunks=None, no_bcast=False, in_split=False,
             o1_dma_engine="sync"):
    import concourse.bass as bass
    import concourse.mybir as mybir
    from concourse.tile import TileContext

    f16 = mybir.dt.float16
    MUL = mybir.AluOpType.mult
    ADD = mybir.AluOpType.add
    COPY = mybir.ActivationFunctionType.Copy
    chunks = chunks or _CHUNKS
    assert sum(chunks) == _FTOT

    nc = bass.Bass("TRN2", target_bir_lowering=False, debug=False)
    x = nc.dram_tensor("xin", [128, _FTOT * _NC_IN], f16, kind="ExternalInput")
    o = nc.dram_tensor("o12", [128, _FTOT * _NC_OUT], f16, kind="ExternalOutput")

    with TileContext(nc) as tc:
        with (
            tc.tile_pool(name="inp", bufs=in_bufs) as inp,
            tc.tile_pool(name="out", bufs=out_bufs) as outp,
            tc.tile_pool(name="tmp", bufs=tmp_bufs) as tmp,
        ):
            engines = {"sync": nc.sync, "gpsimd": nc.gpsimd, "scalar": nc.scalar}
            in_eng = engines[in_dma_engine]
            out_eng = engines[out_dma_engine]

            def emit_chunk(off, F):
                ioff, ooff = _NC_IN * off, _NC_OUT * off
                T = inp.tile([128, _NC_IN * F], f16, tag="tin")
                if in_split:
                    h10 = 10 * F
                    nc.sync.dma_start(T[:, 0:h10], x[:, ioff:ioff + h10])
                    nc.scalar.dma_start(
                        T[:, h10:], x[:, ioff + h10:ioff + _NC_IN * F])
                else:
                    in_eng.dma_start(T[:], x[:, ioff:ioff + _NC_IN * F])
                if dma_only:
                    # ablation: identical HBM traffic, no compute
                    out_eng.dma_start(
                        o[:, ooff:ooff + _NC_OUT * F], T[:, 0:_NC_OUT * F])
                    return
                Tv = T[:].rearrange("p (c f) -> p c f", f=F)

                # pv blocks (11): 0-3 h, reused for u1 once pm consumes h |
                # 4-7 pm (becomes [p1_1..p1_4] in place) | 8-9 u2 | 10 u3
                pv = tmp.tile([128, 11 * F], f16, tag="pv")
                pvv = pv[:].rearrange("p (c f) -> p c f", f=F)
                # qs blocks (8): 0-4 q40..q04 (1-4 become QM in place) | 5-7 S2-4
                qs = tmp.tile([128, 8 * F], f16, tag="qs")
                qv = qs[:].rearrange("p (c f) -> p c f", f=F)
                vq = tmp.tile([128, 3 * F], f16, tag="vq")
                vqv = vq[:].rearrange("p (c f) -> p c f", f=F)
                o2t = outp.tile([128, 4 * F], f16, tag="o2t")
                o2v = o2t[:].rearrange("p (c f) -> p c f", f=F)

                # ---- ACT: q = 1 - [p40 p31 p22 p13 p04] (issued first so it
                # overlaps the DVE p1 head block) ----
                nc.scalar.activation(qv[:, 0:5], Tv[:, 0:5], COPY,
                                     bias=1.0, scale=-1.0)

                # ---- p1 head products (DVE; only need T) ----
                # h = L*M = [p40*m31, p31*m22, p22*m13, p13*m04]
                nc.vector.tensor_tensor(pvv[:, 0:4], Tv[:, 0:4], Tv[:, 5:9], MUL)
                # masked head terms pm = h*[p41 p32 p23 p14]; pm0 = p1_1
                nc.vector.tensor_tensor(pvv[:, 4:8], pvv[:, 0:4], Tv[:, 9:13], MUL)
                # u1 = p40*[p42 p43 p44] over the spent h blocks
                # (no_bcast: timing-only diagnostic for whether stride-0
                # broadcast operands silently drop the DVE to 1x mode)
                u1_rhs = (Tv[:, 0:3] if no_bcast
                          else Tv[:, 0:1].broadcast_to((128, 3, F)))
                nc.vector.tensor_tensor(pvv[:, 0:3], Tv[:, 13:16], u1_rhs, MUL)

                # ---- p2 cumprod chain (ACT q ran during h/pm/u1); issued
                # early so ACT's V op completes long before o2 consumes it ----
                nc.vector.tensor_tensor(qv[:, 5:6], qv[:, 0:1], qv[:, 1:2], MUL)
                nc.vector.tensor_tensor(qv[:, 6:7], qv[:, 5:6], qv[:, 2:3], MUL)
                nc.vector.tensor_tensor(qv[:, 7:8], qv[:, 6:7], qv[:, 3:4], MUL)
                # V = 1 - [S2 S3 S4] on ACT (1-S1 = p40 is a raw input column)
                nc.scalar.activation(vqv[:, 0:3], qv[:, 5:8], COPY,
                                     bias=1.0, scale=-1.0)

                # ---- p1 tail (covers ACT V latency) ----
                # u2 = p31*[p33 p34], u3 = p22*p24
                u2_rhs = (Tv[:, 1:3] if no_bcast
                          else Tv[:, 1:2].broadcast_to((128, 2, F)))
                nc.vector.tensor_tensor(pvv[:, 8:10], Tv[:, 16:18], u2_rhs, MUL)
                nc.vector.tensor_tensor(pvv[:, 10:11], Tv[:, 18:19], Tv[:, 2:3], MUL)
                # p1_j = pm_j + prefix-chain partials via in-place shifted adds;
                # pv[4:8] ends as [p1_1 p1_2 p1_3 p1_4]
                nc.vector.tensor_tensor(pvv[:, 5:8], pvv[:, 5:8], pvv[:, 0:3], ADD)
                nc.vector.tensor_tensor(pvv[:, 6:8], pvv[:, 6:8], pvv[:, 8:10], ADD)
                nc.vector.tensor_tensor(pvv[:, 7:8], pvv[:, 7:8], pvv[:, 10:11], ADD)
                # QM = [q31 q22 q13 q04]*[m31 m22 m13 m04] in place over q
                # (the chain has already consumed q31/q22/q13)
                nc.vector.tensor_tensor(qv[:, 1:5], qv[:, 1:5], Tv[:, 5:9], MUL)
                # p2_1 = (1-S1)*q31*m31 = p40*QM0
                nc.vector.tensor_tensor(o2v[:, 0:1], Tv[:, 0:1], qv[:, 1:2], MUL)
                # p2_j = V_j*QM_j, j=2..4
                nc.vector.tensor_tensor(o2v[:, 1:4], vqv[:, 0:3], qv[:, 2:5], MUL)

                # p1 goes out on the sync(SP) ring, p2 on out_eng: the rings'
                # sequencers block on each DMA's wait-for-DVE sem, so putting
                # both on ACT would delay the next chunk's q activation
                engines[o1_dma_engine].dma_start(
                    o[:, ooff:ooff + 4 * F], pv[:, 4 * F:8 * F])
                out_eng.dma_start(o[:, ooff + 4 * F:ooff + 8 * F], o2t[:])

            for _ in range(reps):
                off = 0
                for F in chunks:
                    emit_chunk(off, F)
                    off += F
    return _legalize_waits(nc) if legalize else nc


def _pack_inputs(output, label_mask):
    """[B,25] f32 x2 -> per-core [128, FTOT*19] fp16 device layout."""
    ntot = _NCORES * _NPC
    xp = np.zeros((ntot, _NC_IN), np.float16)
    xp[:_B, 0:5] = output[:, _PRED_COLS]
    xp[:_B, 5:9] = label_mask[:, _MASK_COLS]
    xp[:_B, 9:19] = output[:, _PRED_COLS2]
    cores = []
    for c in range(_NCORES):
        a = xp[c * _NPC:(c + 1) * _NPC].reshape(128, _FTOT, _NC_IN)
        parts, off = [], 0
        for F in _CHUNKS:
            blk = a[:, off:off + F, :].transpose(0, 2, 1)  # [128, 19, F]
            parts.append(np.ascontiguousarray(blk).reshape(128, _NC_IN * F))
            off += F
        cores.append(np.concatenate(parts, axis=1))
    return cores


def _unpack_outputs(res):
    """Per-core [128, FTOT*8] fp16 -> (p1, p2) [B,5] f32."""
    p1 = np.zeros((_B, 5), np.float32)
    p2 = np.zeros((_B, 5), np.float32)
    rows = np.empty((_NCORES * _NPC, _NC_OUT), np.float32)
    for c in range(_NCORES):
        a = np.asarray(res[c]["o12"])
        parts, off = [], 0
        for F in _CHUNKS:
            blk = a[:, _NC_OUT * off:_NC_OUT * (off + F)]
            parts.append(blk.reshape(128, _NC_OUT, F).transpose(0, 2, 1))
            off += F
        core_rows = np.concatenate(parts, axis=1)  # [128, FTOT, 8]
        rows[c * _NPC:(c + 1) * _NPC] = core_rows.reshape(_NPC, _NC_OUT)
    p1[:, 1:5] = rows[:_B, 0:4]
    p2[:, 1:5] = rows[:_B, 4:8]
    return p1, p2


def _run(output, label_mask, **spmd_kwargs):
    from concourse.bass_utils import run_bass_kernel_spmd

    output = np.ascontiguousarray(np.asarray(output), dtype=np.float32)
    label_mask = np.ascontiguousarray(np.asarray(label_mask), dtype=np.float32)
    assert output.shape == (_B, 25) and label_mask.shape == (_B, 25)

    in_maps = [{"xin": xc} for xc in _pack_inputs(output, label_mask)]
    nc = build_nc()
    bres = run_bass_kernel_spmd(nc, in_maps, list(range(_NCORES)), **spmd_kwargs)
    p1, p2 = _unpack_outputs(bres.results)
    return p1, p2, bres


def kernel(output, label_mask):
    p1, p2, _ = _run(output, label_mask)
    return p1, p2
